# revision 7
# baseline (speedup 1.0000x reference)
"""Trainium2 Bass kernel for nn_CMLITargetLoss (CMLI target loss).

Strategy (pure data parallel, batch 128 -> 16 samples per core x 8 cores):
  Per sample (on-device, bf16 inputs cast during DMA, fp32 accumulation):
    - target^T via DMA-transpose (bf16), rsq[n] = sum_d target^2 via
      squares + ones-matmul column reduce on the PE (fp32 psum accumulate)
    - G[t,n] = text . target via PE matmul (bf16 in, fp32 psum out),
      two samples stacked into one [128,197] psum tile
    - s = G * rinv[n] (rinv broadcast via tiny PE matmul), row-max m,
      mask = (s >= m), v = rsq + C - 2G, vsel = max(mask*v) - C
      => tok_sq[t] = ||text_t||^2 + rsq[n*] - 2*G[t,n*]  (n* = argmax_n s)
    - image loss partials: (image-target)^2 summed over patches via PE
      ones-matmul into a persistent psum row accumulator
  Host: combines 8 cores' partial sums in float64 and applies the final
  normalizations (exactly mirrors the reference formula).

Outputs per core: out_cols [128,4] f32 (col0 masked tok_sq partials,
col1 keep partials, col2 rows 0:16 cls partials), out_row [1,768] f32
(image-loss column partials).
"""

import numpy as np

B, T, N, D = 128, 64, 197, 768
NCORES = 8
BL = B // NCORES  # 16 samples per core
PAIRS = BL // 2
C_OFF = float(2.0**20)
CW = 208  # padded column width for transposed target (197 -> 208)

_CACHE = {}


def _build(n_loop=1):
    from contextlib import ExitStack

    import concourse.bass as bass
    import concourse.tile as tile
    from concourse import bacc, mybir

    f32 = mybir.dt.float32
    bf16 = mybir.dt.bfloat16
    i32 = mybir.dt.int32
    Alu = mybir.AluOpType
    Act = mybir.ActivationFunctionType
    X = mybir.AxisListType.X

    nc = bacc.Bacc("TRN2", target_bir_lowering=False, debug=False)

    image_d = nc.dram_tensor("image", [BL, N, D], f32, kind="ExternalInput").ap()
    text_d = nc.dram_tensor("text", [BL, T, D], f32, kind="ExternalInput").ap()
    target_d = nc.dram_tensor("target", [BL, N, D], f32, kind="ExternalInput").ap()
    pm_d = nc.dram_tensor("pm", [BL, T], i32, kind="ExternalInput").ap()
    id16_d = nc.dram_tensor("id16", [16, 16], f32, kind="ExternalInput").ap()
    out_cols_d = nc.dram_tensor("out_cols", [128, 4], f32, kind="ExternalOutput").ap()
    out_row_d = nc.dram_tensor("out_row", [1, D], f32, kind="ExternalOutput").ap()

    with tile.TileContext(nc) as tc, ExitStack() as ctx:
        cp = ctx.enter_context(tc.tile_pool(name="const", bufs=1))
        ld = ctx.enter_context(tc.tile_pool(name="ld", bufs=3))
        ldi = ctx.enter_context(tc.tile_pool(name="ldi", bufs=2))
        xtp = ctx.enter_context(tc.tile_pool(name="xtp", bufs=2))
        tTp = ctx.enter_context(tc.tile_pool(name="tTp", bufs=3))
        sqp = ctx.enter_context(tc.tile_pool(name="sqp", bufs=2))
        xTp = ctx.enter_context(tc.tile_pool(name="xTp", bufs=2))
        rowp = ctx.enter_context(tc.tile_pool(name="rowp", bufs=3))
        sbk = ctx.enter_context(tc.tile_pool(name="sbk", bufs=2))
        dfp = ctx.enter_context(tc.tile_pool(name="dfp", bufs=2))
        kp = ctx.enter_context(tc.tile_pool(name="kp", bufs=1))
        psA = ctx.enter_context(
            tc.tile_pool(name="psA", bufs=1, space=bass.MemorySpace.PSUM)
        )
        psG = ctx.enter_context(
            tc.tile_pool(name="psG", bufs=2, space=bass.MemorySpace.PSUM)
        )
        psB = ctx.enter_context(
            tc.tile_pool(name="psB", bufs=1, space=bass.MemorySpace.PSUM)
        )
        psS = ctx.enter_context(
            tc.tile_pool(name="psS", bufs=3, space=bass.MemorySpace.PSUM)
        )

        # constants
        ones_bf = cp.tile([128, 1], bf16)
        nc.vector.memset(ones_bf[:], 1.0)
        ones64 = cp.tile([1, 64], f32)
        nc.vector.memset(ones64[:], 1.0)
        ones11 = cp.tile([1, 1], f32)
        nc.vector.memset(ones11[:], 1.0)
        id16_t = cp.tile([16, 16], f32)
        nc.sync.dma_start(id16_t[:], id16_d[:])
        tok_buf = cp.tile([128, PAIRS], f32)
        outc = cp.tile([128, 4], f32)
        outr = cp.tile([1, D], f32)

        target_flat = target_d.rearrange("b n d -> (b n) d")

        def body():
            nc.vector.memset(outc[:], 0.0)

            # persistent image-loss accumulators
            imgacc1 = psA.tile([1, 512], f32, tag="imgacc1")
            imgacc2 = psA.tile([1, 256], f32, tag="imgacc2")

            for p in range(PAIRS):
                xt = xtp.tile([128, D], bf16, tag="xt")
                tTs = []
                for j in range(2):
                    b = 2 * p + j
                    # ---- cast loads (fp32 DRAM -> bf16 SBUF, SWDGE) ----
                    tgt_a = ld.tile([128, D], bf16, tag="tgt_a")
                    nc.gpsimd.dma_start(tgt_a[:], target_d[b, 0:128, :])
                    tgt_b = ld.tile([80, D], bf16, tag="tgt_b")
                    if b < BL - 1:
                        # pad rows 69:80 with the neighbor sample's first rows
                        # (transposed into columns that are sliced out later)
                        nc.gpsimd.dma_start(
                            tgt_b[:], target_flat[N * b + 128 : N * b + 208, :]
                        )
                    else:
                        nc.vector.memset(tgt_b[64:80, :], 0.0)
                        nc.gpsimd.dma_start(tgt_b[0:69, :], target_d[b, 128:197, :])
                    img_a = ldi.tile([128, D], bf16, tag="img_a")
                    nc.gpsimd.dma_start(img_a[:], image_d[b, 0:128, :])
                    img_b = ldi.tile([80, D], bf16, tag="img_b")
                    nc.gpsimd.dma_start(img_b[0:69, :], image_d[b, 128:197, :])
                    nc.gpsimd.dma_start(xt[64 * j : 64 * (j + 1), :], text_d[b, :, :])

                    # ---- target transpose (DMA xbar, bf16) ----
                    tT = tTp.tile([128, 6 * CW], bf16, tag="tT")
                    for c in range(6):
                        nc.sync.dma_start(
                            tT[:, CW * c : CW * c + 128],
                            tgt_a[:, 128 * c : 128 * (c + 1)],
                            transpose=True,
                        )
                        nc.sync.dma_start(
                            tT[:, CW * c + 128 : CW * c + 208],
                            tgt_b[:, 128 * c : 128 * (c + 1)],
                            transpose=True,
                        )
                    tTs.append(tT)

                    # ---- rsq[n] = sum_d target^2 ----
                    sq = sqp.tile([128, 6 * CW], bf16, tag="sq")
                    nc.vector.tensor_tensor(
                        sq[:, 0:624], tT[:, 0:624], tT[:, 0:624], Alu.mult
                    )
                    nc.gpsimd.tensor_tensor(
                        sq[:, 624:1248], tT[:, 624:1248], tT[:, 624:1248], Alu.mult
                    )
                    rsq = psS.tile([1, CW], f32, tag="small")
                    for c in range(6):
                        nc.tensor.matmul(
                            rsq[:],
                            ones_bf[:],
                            sq[:, CW * c : CW * (c + 1)],
                            start=(c == 0),
                            stop=(c == 5),
                        )
                    r_row = rowp.tile([1, CW], f32, tag="r_row")
                    nc.scalar.activation(r_row[:, 0:197], rsq[:, 0:197], Act.Sqrt)
                    rinv_row = rowp.tile([1, CW], f32, tag="rinv_row")
                    nc.vector.reciprocal(rinv_row[:, 0:197], r_row[:, 0:197])
                    rsqC_row = rowp.tile([1, CW], f32, tag="rsqC_row")
                    nc.scalar.activation(
                        rsqC_row[:, 0:197], rsq[:, 0:197], Act.Copy, bias=C_OFF
                    )

                    # ---- broadcasts into psum [128, 416]: rinv | rsqC ----
                    if j == 0:
                        bc = psB.tile([128, 2 * CW], f32, tag="bc")
                    nc.tensor.matmul(
                        bc[64 * j : 64 * (j + 1), 0:197],
                        ones64[:],
                        rinv_row[:, 0:197],
                        start=True,
                        stop=True,
                    )
                    nc.tensor.matmul(
                        bc[64 * j : 64 * (j + 1), CW : CW + 197],
                        ones64[:],
                        rsqC_row[:, 0:197],
                        start=True,
                        stop=True,
                    )

                    # ---- image loss: (image - target)^2 ----
                    diff_a = dfp.tile([128, D], bf16, tag="diff_a")
                    nc.vector.tensor_tensor(diff_a[:], img_a[:], tgt_a[:], Alu.subtract)
                    diff_b = dfp.tile([80, D], bf16, tag="diff_b")
                    nc.gpsimd.tensor_tensor(
                        diff_b[0:69, :], img_b[0:69, :], tgt_b[0:69, :], Alu.subtract
                    )
                    dsq_a = dfp.tile([128, D], bf16, tag="dsq_a")
                    nc.scalar.activation(dsq_a[:], diff_a[:], Act.Square)
                    dsq_b = dfp.tile([80, D], bf16, tag="dsq_b")
                    nc.vector.tensor_tensor(
                        dsq_b[0:69, :], diff_b[0:69, :], diff_b[0:69, :], Alu.mult
                    )
                    first = b == 0
                    last = b == BL - 1
                    nc.tensor.matmul(
                        imgacc1[:], ones_bf[:], dsq_a[:, 0:512],
                        start=first, stop=False, skip_group_check=True,
                    )
                    nc.tensor.matmul(
                        imgacc2[:], ones_bf[:], dsq_a[:, 512:768],
                        start=first, stop=False, skip_group_check=True,
                    )
                    nc.tensor.matmul(
                        imgacc1[:], ones_bf[0:69, :], dsq_b[0:69, 0:512],
                        start=False, stop=False, skip_group_check=True,
                    )
                    nc.tensor.matmul(
                        imgacc2[:], ones_bf[0:69, :], dsq_b[0:69, 512:768],
                        start=False, stop=last, skip_group_check=True,
                    )

                # ---- text transpose for the pair ----
                xT = xTp.tile([128, D], bf16, tag="xT")
                for c in range(6):
                    nc.sync.dma_start(
                        xT[:, 128 * c : 128 * (c + 1)],
                        xt[:, 128 * c : 128 * (c + 1)],
                        transpose=True,
                    )

                # ---- G = text . target (pair-stacked [128, 197] psum) ----
                G = psG.tile([128, CW], f32, tag="G")
                for j in range(2):
                    for c in range(6):
                        nc.tensor.matmul(
                            G[64 * j : 64 * (j + 1), 0:197],
                            xT[:, 128 * c + 64 * j : 128 * c + 64 * (j + 1)],
                            tTs[j][:, CW * c : CW * c + 197],
                            start=(c == 0),
                            stop=(c == 5),
                        )

                # ---- selection block ----
                G_sb = sbk.tile([128, CW], f32, tag="G_sb")
                nc.vector.tensor_copy(G_sb[:, 0:197], G[:, 0:197])
                s = sbk.tile([128, CW], f32, tag="s")
                nc.vector.tensor_tensor(
                    s[:, 0:197], G_sb[:, 0:197], bc[:, 0:197], Alu.mult
                )
                m = sbk.tile([128, 1], f32, tag="m")
                nc.vector.tensor_reduce(m[:], s[:, 1:197], X, Alu.max)
                v = sbk.tile([128, CW], f32, tag="v")
                nc.vector.scalar_tensor_tensor(
                    v[:, 0:196], G_sb[:, 1:197], -2.0, bc[:, CW + 1 : CW + 197],
                    op0=Alu.mult, op1=Alu.add,
                )
                y = sbk.tile([128, CW], f32, tag="y")
                nc.vector.scalar_tensor_tensor(
                    y[:, 0:196], s[:, 1:197], m[:], v[:, 0:196],
                    op0=Alu.is_ge, op1=Alu.mult,
                )
                vsel = sbk.tile([128, 1], f32, tag="vsel")
                nc.vector.tensor_reduce(vsel[:], y[:, 0:196], X, Alu.max)

                # ---- textsq[t] = sum_d text^2, as a column ----
                sqx = xTp.tile([128, D], bf16, tag="sqx")
                nc.vector.tensor_tensor(sqx[:], xT[:], xT[:], Alu.mult)
                tsq_row = psS.tile([1, 128], f32, tag="small")
                for c in range(6):
                    nc.tensor.matmul(
                        tsq_row[:],
                        ones_bf[:],
                        sqx[:, 128 * c : 128 * (c + 1)],
                        start=(c == 0),
                        stop=(c == 5),
                    )
                tsq_sb = rowp.tile([1, 128], f32, tag="tsq_sb")
                nc.vector.tensor_copy(tsq_sb[:], tsq_row[:])
                tsqT = psS.tile([128, 1], f32, tag="small")
                nc.tensor.matmul(tsqT[:], tsq_sb[:], ones11[:], start=True, stop=True)

                # tok_sq column for this pair: textsq + (vsel - C)
                nc.vector.scalar_tensor_tensor(
                    tok_buf[:, p : p + 1], vsel[:], -C_OFF, tsqT[:],
                    op0=Alu.add, op1=Alu.add,
                )

            # ---- keep mask ----
            pm_t = kp.tile([BL, T], i32, tag="pm_t")
            nc.sync.dma_start(pm_t[:], pm_d[:])
            pmf = kp.tile([BL, T], f32, tag="pmf")
            nc.vector.tensor_copy(pmf[:], pm_t[:])
            pmT = psS.tile([T, BL], f32, tag="small")
            nc.tensor.matmul(pmT[:], pmf[:], id16_t[:], start=True, stop=True)
            kT = kp.tile([128, PAIRS], f32, tag="kT")
            pmT3 = pmT[:].rearrange("p (e two) -> p two e", two=2)
            nc.vector.tensor_copy(kT[0:64, :], pmT3[:, 0, :])
            nc.vector.tensor_copy(kT[64:128, :], pmT3[:, 1, :])
            keep = kp.tile([128, PAIRS], f32, tag="keep")
            nc.vector.tensor_scalar(keep[:], kT[:], 0.0, None, op0=Alu.is_equal)
            nc.vector.memset(keep[0:1, :], 0.0)
            nc.vector.memset(keep[64:65, :], 0.0)

            junk = kp.tile([128, PAIRS], f32, tag="junk")
            nc.vector.scalar_tensor_tensor(
                junk[:], tok_buf[:], 1.0, keep[:], op0=Alu.mult, op1=Alu.mult,
                accum_out=outc[:, 0:1],
            )
            nc.vector.tensor_reduce(outc[:, 1:2], keep[:], X, Alu.add)

            # ---- cls term ----
            tcls = kp.tile([BL, D], bf16, tag="tcls")
            nc.gpsimd.dma_start(tcls[:], text_d[:, 0, :])
            icls = kp.tile([BL, D], bf16, tag="icls")
            nc.gpsimd.dma_start(icls[:], image_d[:, 0, :])
            dcls = kp.tile([BL, D], bf16, tag="dcls")
            nc.vector.tensor_tensor(dcls[:], tcls[:], icls[:], Alu.subtract)
            cjunk = kp.tile([BL, D], f32, tag="cjunk")
            nc.vector.scalar_tensor_tensor(
                cjunk[:], dcls[:], 1.0, dcls[:], op0=Alu.mult, op1=Alu.mult,
                accum_out=outc[0:BL, 2:3],
            )

            # ---- image loss rows out ----
            nc.vector.tensor_copy(outr[:, 0:512], imgacc1[:])
            nc.vector.tensor_copy(outr[:, 512:768], imgacc2[:])

            nc.sync.dma_start(out_cols_d[:], outc[:])
            nc.sync.dma_start(out_row_d[:], outr[:])

        if n_loop > 1:
            with tc.For_i(0, n_loop, 1):
                body()
        else:
            body()

    nc.compile()
    return nc


def _get_nc(n_loop=1):
    if n_loop not in _CACHE:
        _CACHE[n_loop] = _build(n_loop)
    return _CACHE[n_loop]


def _run(nc, image, text, target, padding_mask):
    from concourse.bass_utils import run_bass_kernel_spmd

    image = np.ascontiguousarray(np.asarray(image, dtype=np.float32))
    text = np.ascontiguousarray(np.asarray(text, dtype=np.float32))
    target = np.ascontiguousarray(np.asarray(target, dtype=np.float32))
    pm = np.ascontiguousarray(np.asarray(padding_mask, dtype=np.int32))
    id16 = np.eye(16, dtype=np.float32)

    in_maps = []
    for c in range(NCORES):
        sl = slice(c * BL, (c + 1) * BL)
        in_maps.append(
            {
                "image": image[sl],
                "text": text[sl],
                "target": target[sl],
                "pm": pm[sl],
                "id16": id16,
            }
        )
    res = run_bass_kernel_spmd(nc, in_maps, list(range(NCORES)))
    return res


def _combine(results):
    masked = 0.0
    keep = 0.0
    cls = 0.0
    img = 0.0
    for r in results:
        oc = r["out_cols"].astype(np.float64)
        orow = r["out_row"].astype(np.float64)
        masked += oc[:, 0].sum()
        keep += oc[:, 1].sum()
        cls += oc[0:BL, 2].sum()
        img += orow.sum()
    kd_text = (cls + masked) / ((B + keep) * D)
    kd_img = img / (B * N * D)
    return np.asarray((kd_text + kd_img) / 2.0, dtype=np.float32)


def kernel(image, text, target, padding_mask):
    nc = _get_nc(1)
    res = _run(nc, image, text, target, padding_mask)
    return _combine(res.results)
